# revision 10
# baseline (speedup 1.0000x reference)
"""Bass/Trainium2 kernel for nn_EntangleComplex.

The reference computes (x_real @ op, x_imag @ op) where op is a DIAGONAL
matrix with +-1 entries (elementwise product of diagonal CZ-style gates).
Hence x @ op == x * diag(op)[None, :] exactly (IEEE: off-diagonal terms
are exact zeros).  The device kernel is therefore a DMA-bound elementwise
multiply by a broadcast sign vector, data-parallel over the batch dim
across 8 NeuronCores with no communication.

Per core: 512 rows of x_real + 512 rows of x_imag (16 MiB in, 16 MiB
out).  The sign vector is DMA'd as one 8 KiB bf16 row and broadcast to
all 128 SBUF partitions with K=1 bf16 PE matmuls against a ones vector
(exact for +-1), so DMA traffic stays at the 32 MiB roofline.

Raw Bass (no Tile) with explicit semaphores: loads on the SP HWDGE ring,
stores on the Activation HWDGE ring (a store's semaphore wait must never
block load issue), multiplies on DVE.  1 MiB strips keep reads and
writes mixed on the HBM stack (pure-read phases cap ~100 GB/s lower per
NC than mixed) and shorten the store-only tail.  No Tile entry barrier
or end-of-kernel drain/EVSEM butterfly.
"""

from contextlib import ExitStack

import numpy as np
import ml_dtypes

import concourse.bacc as bacc
import concourse.mybir as mybir
from concourse.bass_utils import run_bass_kernel_spmd

N_CORES = 8
BATCH = 4096
DIM = 4096
ROWS = BATCH // N_CORES  # 512 rows of each of x_real/x_imag per core
P = 128                  # SBUF partition count
MM_N = 512               # PSUM bank free-dim limit per matmul
NJ = DIM // MM_N         # 8 broadcast chunks
SW = 2048                # strip width (1 MiB strips)
NSC = DIM // SW          # col-strips per row-tile
NRT = 2 * ROWS // P      # row-tiles of [128, DIM] per core (8)
NS = NRT * NSC           # strips per core (16)

_NC = None


def _build_program():
    global _NC
    if _NC is not None:
        return _NC
    nc = bacc.Bacc(enable_partition_id=False)
    f32 = mybir.dt.float32
    bf16 = mybir.dt.bfloat16
    xr = nc.declare_dram_parameter("xr", [ROWS, DIM], f32, isOutput=False)
    xi = nc.declare_dram_parameter("xi", [ROWS, DIM], f32, isOutput=False)
    d = nc.declare_dram_parameter("d", [1, DIM], bf16, isOutput=False)
    yr = nc.declare_dram_parameter("yr", [ROWS, DIM], f32, isOutput=True)
    yi = nc.declare_dram_parameter("yi", [ROWS, DIM], f32, isOutput=True)

    def dram_ap(t_pair, s):
        r, c = divmod(s, NSC)
        t, rr = (t_pair[0], r) if r < NRT // 2 else (t_pair[1], r - NRT // 2)
        return t[rr * P:(rr + 1) * P, c * SW:(c + 1) * SW]

    with ExitStack() as ctx:
        dsmall = ctx.enter_context(nc.sbuf_tensor("dsmall", [1, DIM], bf16))
        ones = ctx.enter_context(nc.sbuf_tensor("ones", [1, P], bf16))
        dtile = ctx.enter_context(nc.sbuf_tensor("dtile", [P, DIM], f32))
        xts = [
            ctx.enter_context(nc.sbuf_tensor(f"xt{s}", [P, SW], f32))
            for s in range(NS)
        ]
        pbs = [
            ctx.enter_context(nc.psum_tensor(f"pb{j}", [P, MM_N], f32))
            for j in range(2)
        ]
        dsem = ctx.enter_context(nc.semaphore("dsem"))
        osem = ctx.enter_context(nc.semaphore("osem"))
        mmsem = ctx.enter_context(nc.semaphore("mmsem"))
        cpsem = ctx.enter_context(nc.semaphore("cpsem"))
        mulsem = ctx.enter_context(nc.semaphore("mulsem"))
        ssem = ctx.enter_context(nc.semaphore("ssem"))
        lsems = [ctx.enter_context(nc.semaphore(f"lsem{s}")) for s in range(NS)]
        block = ctx.enter_context(nc.Block())

        @block.sync
        def _(sync):
            sync.dma_start(dsmall[:], d[:]).then_inc(dsem, 16)
            for s in range(NS):
                sync.dma_start(xts[s][:], dram_ap((xr, xi), s)).then_inc(
                    lsems[s], 16
                )

        @block.tensor
        def _(tensor):
            tensor.wait_ge(osem, 1)
            tensor.wait_ge(dsem, 16)
            for j in range(NJ):
                if j >= 2:
                    # PSUM WAR: bank j%2 must have been copied out
                    tensor.wait_ge(cpsem, j - 1)
                nc.tensor.matmul(
                    pbs[j % 2][:],
                    ones[:],
                    dsmall[0:1, j * MM_N:(j + 1) * MM_N],
                    start=True,
                    stop=True,
                ).then_inc(mmsem, 1)

        @block.vector
        def _(vector):
            vector.memset(ones[:], 1.0).then_inc(osem, 1)
            for j in range(NJ):
                vector.wait_ge(mmsem, j + 1)
                vector.tensor_copy(
                    dtile[:, j * MM_N:(j + 1) * MM_N], pbs[j % 2][:]
                ).then_inc(cpsem, 1)
            # deep-pipeline RAW: muls read dtile, so wait for the copies'
            # writebacks even though they ran on this same engine
            vector.wait_ge(cpsem, NJ)
            for s in range(NS):
                c = s % NSC
                vector.wait_ge(lsems[s], 16)
                vector.tensor_mul(
                    xts[s][:], xts[s][:], dtile[:, c * SW:(c + 1) * SW]
                ).then_inc(mulsem, 1)

        @block.scalar
        def _(scalar):
            for s in range(NS):
                scalar.wait_ge(mulsem, s + 1)
                scalar.dma_start(dram_ap((yr, yi), s), xts[s][:]).then_inc(
                    ssem, 16
                )
            # outputs are in HBM once every store's sem receipt fired
            scalar.wait_ge(ssem, 16 * NS)

    nc.finalize()
    _NC = nc
    return nc


def kernel(x_real, x_imag, op):
    x_real = np.ascontiguousarray(np.asarray(x_real, dtype=np.float32))
    x_imag = np.ascontiguousarray(np.asarray(x_imag, dtype=np.float32))
    op = np.asarray(op, dtype=np.float32)
    dvec = (
        np.ascontiguousarray(np.diagonal(op))
        .astype(ml_dtypes.bfloat16)
        .reshape(1, DIM)
    )

    nc = _build_program()
    in_maps = []
    for c in range(N_CORES):
        sl = slice(c * ROWS, (c + 1) * ROWS)
        in_maps.append({"xr": x_real[sl], "xi": x_imag[sl], "d": dvec})
    res = run_bass_kernel_spmd(nc, in_maps, list(range(N_CORES))).results
    y_real = np.concatenate([r["yr"] for r in res], axis=0)
    y_imag = np.concatenate([r["yi"] for r in res], axis=0)
    return y_real, y_imag


# revision 23
# speedup vs baseline: 1.0315x; 1.0315x over previous
"""Bass/Trainium2 kernel for nn_EntangleComplex.

The reference computes (x_real @ op, x_imag @ op) where op is a DIAGONAL
matrix with +-1 entries (elementwise product of diagonal CZ-style gates).
Hence x @ op == x * diag(op)[None, :] exactly (IEEE: off-diagonal terms
are exact zeros).  The device kernel is therefore a DMA-bound elementwise
multiply by a broadcast sign vector, data-parallel over the batch dim
across 8 NeuronCores with no communication.

Per core: 512 rows of x_real + 512 rows of x_imag (16 MiB in, 16 MiB
out).  The sign vector is DMA'd as one 8 KiB bf16 row and broadcast to
all 128 SBUF partitions with K=1 bf16 PE matmuls against a ones vector
(exact for +-1), so DMA traffic stays at the 32 MiB roofline.

Raw Bass (no Tile) with explicit semaphores: loads on the SP HWDGE ring,
stores + the d row on the Activation HWDGE ring (a store's semaphore
wait must never block load issue), multiplies on DVE.  Uniform
[128, 2048] f32 strips (1 MiB) — this shape packetizes as 16 KiB DMA
packets which run at full per-engine rate; smaller/unaligned strips
degrade to 2-8 KiB packets at ~70% rate.  The broadcast-chunk copies are
interleaved with the first row-tile's muls so stores start early:
keeping reads and writes mixed matters because the HBM stack shared by
NC pairs serves pure-read phases ~100 GB/s slower per NC than mixed.
"""

from contextlib import ExitStack

import numpy as np
import ml_dtypes

import concourse.bacc as bacc
import concourse.mybir as mybir
from concourse.bass_utils import run_bass_kernel_spmd

N_CORES = 8
BATCH = 4096
DIM = 4096
ROWS = BATCH // N_CORES  # 512 rows of each of x_real/x_imag per core
P = 128                  # SBUF partition count
MM_N = 512               # PSUM bank free-dim limit per matmul
NJ = DIM // MM_N         # 8 broadcast chunks
SW = 2048                # strip width (1 MiB strips, 16 KiB packets)
NSC = DIM // SW          # col-strips per row-tile (2)
NRT = 2 * ROWS // P      # row-tiles of [128, DIM] per core (8)
NS = NRT * NSC           # strips per core (16)
CPS = SW // MM_N         # broadcast chunks per strip (4)

_NC = None


def _build_program():
    global _NC
    if _NC is not None:
        return _NC
    nc = bacc.Bacc(enable_partition_id=False)
    f32 = mybir.dt.float32
    bf16 = mybir.dt.bfloat16
    xr = nc.declare_dram_parameter("xr", [ROWS, DIM], f32, isOutput=False)
    xi = nc.declare_dram_parameter("xi", [ROWS, DIM], f32, isOutput=False)
    d = nc.declare_dram_parameter("d", [1, DIM], bf16, isOutput=False)
    yr = nc.declare_dram_parameter("yr", [ROWS, DIM], f32, isOutput=True)
    yi = nc.declare_dram_parameter("yi", [ROWS, DIM], f32, isOutput=True)

    def dram_ap(t_pair, s):
        r, c = divmod(s, NSC)
        t, rr = (t_pair[0], r) if r < NRT // 2 else (t_pair[1], r - NRT // 2)
        return t[rr * P:(rr + 1) * P, c * SW:(c + 1) * SW]

    with ExitStack() as ctx:
        dsmall = ctx.enter_context(nc.sbuf_tensor("dsmall", [1, DIM], bf16))
        ones = ctx.enter_context(nc.sbuf_tensor("ones", [1, P], bf16))
        dtile = ctx.enter_context(nc.sbuf_tensor("dtile", [P, DIM], f32))
        xts = [
            ctx.enter_context(nc.sbuf_tensor(f"xt{s}", [P, SW], f32))
            for s in range(NS)
        ]
        pbs = [
            ctx.enter_context(nc.psum_tensor(f"pb{j}", [P, MM_N], f32))
            for j in range(2)
        ]
        dsem = ctx.enter_context(nc.semaphore("dsem"))
        osem = ctx.enter_context(nc.semaphore("osem"))
        mmsem = ctx.enter_context(nc.semaphore("mmsem"))
        cpsem = ctx.enter_context(nc.semaphore("cpsem"))
        mulsem = ctx.enter_context(nc.semaphore("mulsem"))
        ssem = ctx.enter_context(nc.semaphore("ssem"))
        lsems = [ctx.enter_context(nc.semaphore(f"lsem{s}")) for s in range(NS)]
        block = ctx.enter_context(nc.Block())

        @block.sync
        def _(sync):
            for s in range(NS):
                sync.dma_start(xts[s][:], dram_ap((xr, xi), s)).then_inc(
                    lsems[s], 16
                )

        @block.tensor
        def _(tensor):
            tensor.wait_ge(osem, 1)
            tensor.wait_ge(dsem, 16)
            for j in range(NJ):
                if j >= 2:
                    # PSUM WAR: bank j%2 must have been copied out
                    tensor.wait_ge(cpsem, j - 1)
                nc.tensor.matmul(
                    pbs[j % 2][:],
                    ones[:],
                    dsmall[0:1, j * MM_N:(j + 1) * MM_N],
                    start=True,
                    stop=True,
                ).then_inc(mmsem, 1)

        def mul_strip(vector, s):
            c = s % NSC
            vector.wait_ge(lsems[s], 16)
            vector.tensor_mul(
                xts[s][:], xts[s][:], dtile[:, c * SW:(c + 1) * SW]
            ).then_inc(mulsem, 1)

        @block.vector
        def _(vector):
            vector.memset(ones[:], 1.0).then_inc(osem, 1)
            # interleave broadcast-chunk copies with row-tile-0 strip muls:
            # strip (0, c) only needs chunks [c*CPS, (c+1)*CPS), so its mul
            # (and store) can run while later chunks are still materializing.
            # The first strip is multiplied chunk-by-chunk right behind the
            # copies so store 0 issues as early as possible.
            for j in range(CPS):
                vector.wait_ge(mmsem, j + 1)
                vector.tensor_copy(
                    dtile[:, j * MM_N:(j + 1) * MM_N], pbs[j % 2][:]
                ).then_inc(cpsem, 1)
                # deep-pipeline RAW on this same engine: wait for the
                # copy's writeback before the mul reads dtile
                vector.wait_ge(cpsem, j + 1)
                if j == 0:
                    vector.wait_ge(lsems[0], 16)
                mm = vector.tensor_mul(
                    xts[0][:, j * MM_N:(j + 1) * MM_N],
                    xts[0][:, j * MM_N:(j + 1) * MM_N],
                    dtile[:, j * MM_N:(j + 1) * MM_N],
                )
                if j == CPS - 1:
                    # in-order completion: the last sub-mul finishing means
                    # all of strip 0 is multiplied
                    mm.then_inc(mulsem, 1)
            for j in range(CPS, NJ):
                vector.wait_ge(mmsem, j + 1)
                vector.tensor_copy(
                    dtile[:, j * MM_N:(j + 1) * MM_N], pbs[j % 2][:]
                ).then_inc(cpsem, 1)
            vector.wait_ge(cpsem, NJ)
            mul_strip(vector, 1)
            for s in range(NSC, NS):
                mul_strip(vector, s)

        @block.scalar
        def _(scalar):
            scalar.dma_start(dsmall[:], d[:]).then_inc(dsem, 16)
            for s in range(NS):
                scalar.wait_ge(mulsem, s + 1)
                scalar.dma_start(dram_ap((yr, yi), s), xts[s][:]).then_inc(
                    ssem, 16
                )
            # outputs are in HBM once every store's sem receipt fired
            scalar.wait_ge(ssem, 16 * NS)

    nc.finalize()
    _NC = nc
    return nc


def kernel(x_real, x_imag, op):
    x_real = np.ascontiguousarray(np.asarray(x_real, dtype=np.float32))
    x_imag = np.ascontiguousarray(np.asarray(x_imag, dtype=np.float32))
    op = np.asarray(op, dtype=np.float32)
    dvec = (
        np.ascontiguousarray(np.diagonal(op))
        .astype(ml_dtypes.bfloat16)
        .reshape(1, DIM)
    )

    nc = _build_program()
    in_maps = []
    for c in range(N_CORES):
        sl = slice(c * ROWS, (c + 1) * ROWS)
        in_maps.append({"xr": x_real[sl], "xi": x_imag[sl], "d": dvec})
    res = run_bass_kernel_spmd(nc, in_maps, list(range(N_CORES))).results
    y_real = np.concatenate([r["yr"] for r in res], axis=0)
    y_imag = np.concatenate([r["yi"] for r in res], axis=0)
    return y_real, y_imag
